# revision 4
# baseline (speedup 1.0000x reference)
"""CRF sequence-score kernel for Trainium2 (8 NeuronCores, SPMD).

Strategy (S-shard: core k owns s in [64k, 64k+64), all 512 batches):
  rows r = s_local*512 + b, laid out as [q = r%128 partitions, x = r//128].
  - emit[r] = emissions[r, tags[r]] via one fused DVE scalar_tensor_tensor
    per 128-row block: accum_out = sum_t (iota_t == tag) * em[r, t].
  - trans[r] = T[tag_r, tagnext_r] via PE chain per block: broadcast-matmul
    of the block's tag row -> transposed one-hot (tensor_scalar vs partition
    iota) -> matmul fetches T rows into PSUM -> same stt selects tagnext.
  - masks folded in a small epilogue; reduction over s via AP-strided
    tensor_reduce; start/end terms via 1-idx-per-partition indirect DMA
    (end term computed exactly: mask column-sum -> last tag gather -> end
    table gather).
Host sums the 8 per-core [128, 4] partials; score[b] = total[b%128, b//128].
"""
import numpy as np

SEQ, BATCH, NTAGS = 512, 512, 128
NCORES = 8
SLICE = SEQ // NCORES            # 64 s-rows per core
NROWS = SLICE * BATCH            # 32768 rows per core
NBLK = NROWS // 128              # 256 blocks of 128 rows
P = 128

_RUNNER = None


# ---------------------------------------------------------------------------
# walrus workaround: this build allows only ONE sync-wait per instruction.
def _install_tile_patch():
    import bass_rust
    import concourse.mybir as mybir
    import concourse.tile as tile
    from concourse.vector_clock import ScopedClock

    if getattr(tile.TileContext, "_crf_patched", False):
        return

    def _drain_and_barrier(self, tick_clock, wait_clock):
        nc = self.nc
        drain_inst = nc.sync.drain()
        wait_clock.add_sem_waits(
            drain_inst.ins, ScopedClock({None: tick_clock.global_clock})
        )
        si = drain_inst.ins.sync_info
        waits = list(si.on_wait) if si is not None and si.on_wait else []
        if len(waits) > 1:
            si.on_wait = waits[:1]
            for w in waits[1:]:
                extra = nc.sync.drain()
                if extra.ins.sync_info is None:
                    extra.ins.sync_info = bass_rust.SyncInfo(on_wait=[], on_update=[])
                extra.ins.sync_info.on_wait = [w]
        nc.all_engine_barrier()
        assert self.sems is not None
        popped = nc._tile_sem_poison_stack.pop()
        assert popped is self._sem_poison
        nc.clear_and_free_semaphores(list(self.sems.allocated().values()))
        nc.all_engine_barrier()

    orig_commit = tile.TileContext._commit_instruction

    def _commit(self, inst, lazy_reg_writes=True):
        si = getattr(inst, "sync_info", None)
        if (
            si is not None
            and si.on_wait
            and len(si.on_wait) > 1
            and inst.engine != mybir.EngineType.Unassigned
        ):
            waits = list(si.on_wait)
            si.on_wait = waits[:1]
            for w in waits[1:]:
                nop = mybir.InstNoOp(name=f"I-{self.nc.next_id()}", ins=[], outs=[])
                nop.engine = inst.engine
                nop.sync_info = bass_rust.SyncInfo(on_wait=[w], on_update=[])
                self._add_instruction(nop)
        return orig_commit(self, inst, lazy_reg_writes)

    tile.TileContext._drain_and_barrier = _drain_and_barrier
    tile.TileContext._commit_instruction = _commit
    tile.TileContext._crf_patched = True


# ---------------------------------------------------------------------------
def _build_nc():
    import concourse.bass as bass
    import concourse.mybir as mybir
    import concourse.tile as tile
    from concourse.masks import make_identity

    F32, I32, BF16, I16 = (mybir.dt.float32, mybir.dt.int32,
                           mybir.dt.bfloat16, mybir.dt.int16)
    AL = mybir.AluOpType

    nc = bass.Bass()
    em = nc.declare_dram_parameter("em", [NROWS * NTAGS], F32, isOutput=False)
    tagx_i = nc.declare_dram_parameter("tagx_i", [NROWS * 2], I32, isOutput=False)
    tagnx_i = nc.declare_dram_parameter("tagnx_i", [NROWS * 2], I32, isOutput=False)
    maskem_i = nc.declare_dram_parameter("maskem_i", [NROWS], I32, isOutput=False)
    masktr_i = nc.declare_dram_parameter("masktr_i", [NROWS], I32, isOutput=False)
    tmat_bf = nc.declare_dram_parameter("tmat_bf", [P, NTAGS], BF16, isOutput=False)
    startv = nc.declare_dram_parameter("startv", [NTAGS, 1], F32, isOutput=False)
    endv = nc.declare_dram_parameter("endv", [NTAGS, 1], F32, isOutput=False)
    maskf_i = nc.declare_dram_parameter("maskf_i", [SEQ * BATCH], I32, isOutput=False)
    tagf_i = nc.declare_dram_parameter("tagf_i", [SEQ * BATCH * 2, 1], I32, isOutput=False)
    out = nc.declare_dram_parameter("out", [P, 4], F32, isOutput=True)

    with tile.TileContext(nc) as tc:
        with tc.tile_pool(name="sbuf", bufs=1) as sb, \
             tc.tile_pool(name="psum", bufs=1, space="PSUM") as ps, \
             tc.tile_pool(name="emp", bufs=3) as emp:
            # ---- constants
            iota_i = sb.tile([P, NTAGS], I32, name="iota_i")
            nc.gpsimd.iota(iota_i[:], pattern=[[1, NTAGS]], base=0, channel_multiplier=0)
            iota = sb.tile([P, NTAGS], F32, name="iota")
            nc.vector.tensor_copy(out=iota[:], in_=iota_i[:])
            iop_i = sb.tile([P, 1], I32, name="iop_i")
            nc.gpsimd.iota(iop_i[:], pattern=[[0, 1]], base=0, channel_multiplier=1)
            iop = sb.tile([P, 1], F32, name="iop")
            nc.vector.tensor_copy(out=iop[:], in_=iop_i[:])
            ones = sb.tile([P, P], F32, name="ones")
            nc.vector.memset(ones[:], 1.0)
            ident = sb.tile([P, P], F32, name="ident")
            make_identity(nc, ident[:])

            # ---- T matrix (bf16) stationary
            tmat = sb.tile([P, NTAGS], BF16, name="tmat")
            nc.sync.dma_start(out=tmat[:], in_=tmat_bf[:])

            # ---- tag/mask staging: x-major loads -> f32 -> PE transpose
            # TAGX[p, sub*128+m] = tag[128*(sub*128+p) + m]
            def stage_tags(name, dram):
                raw = sb.tile([P, 512], I32, name=f"{name}_raw")
                nc.sync.dma_start(
                    out=raw[:].rearrange("p (s i) -> p s i", s=2),
                    in_=dram[:].rearrange("(s p i) -> p s i", s=2, p=P, i=256),
                )
                f = sb.tile([P, 256], F32, name=f"{name}_f")
                nc.vector.tensor_copy(
                    out=f[:].rearrange("p (s m) -> p s m", s=2),
                    in_=raw[:].rearrange("p (s m two) -> p s m two", s=2, two=2)[:, :, :, 0:1],
                )
                return f

            def stage_mask(name, dram):
                raw = sb.tile([P, 256], I32, name=f"{name}_raw")
                nc.sync.dma_start(
                    out=raw[:].rearrange("p (s i) -> p s i", s=2),
                    in_=dram[:].rearrange("(s p i) -> p s i", s=2, p=P, i=P),
                )
                f = sb.tile([P, 256], F32, name=f"{name}_f")
                nc.vector.tensor_copy(out=f[:], in_=raw[:])
                return f

            tagx = stage_tags("tagx", tagx_i)     # [128, 2, 128] f32
            tagnx = stage_tags("tagnx", tagnx_i)
            mex = stage_mask("mex", maskem_i)
            mtx = stage_mask("mtx", masktr_i)

            # transpose halves -> [q, x] layout [128, 256]
            def transpose_qx(name, src):
                dst = sb.tile([P, 256], F32, name=f"{name}_t")
                for h in range(2):
                    tp = ps.tile([P, P], F32, name=f"{name}_tp{h}", tag=f"tb{h}")
                    nc.tensor.transpose(out=tp[:], in_=src[:, h * P:(h + 1) * P],
                                        identity=ident[:])
                    nc.scalar.copy(out=dst[:, h * P:(h + 1) * P], in_=tp[:])
                return dst

            tagt = transpose_qx("tagt", tagx)     # tag in [q, x]
            tagnt = transpose_qx("tagnt", tagnx)  # tagnext in [q, x]
            memt = transpose_qx("memt", mex)      # maskEM in [q, x]
            mtrt = transpose_qx("mtrt", mtx)      # maskTR in [q, x]

            # ---- main loop: emit-stt + trans chain per block x
            eacc = sb.tile([P, 256], F32, name="eacc")
            tacc = sb.tile([P, 256], F32, name="tacc")
            junks = [sb.tile([P, NTAGS], F32, name=f"junk{i}", tag=f"jk{i}")
                     for i in range(8)]
            ohts = [sb.tile([P, P], BF16, name=f"oht{i}", tag=f"oh{i}")
                    for i in range(4)]
            emch = None
            for x in range(NBLK):
                d, sub = x // 16, x % 16
                if sub == 0:
                    emch = emp.tile([P, 16 * NTAGS], F32, name=f"emch{d}", tag="emch")
                    nc.sync.dma_start(
                        out=emch[:].rearrange("p (s t) -> p s t", s=16),
                        in_=em[d * 16 * 16384:(d + 1) * 16 * 16384].rearrange(
                            "(s p t) -> p s t", s=16, p=P, t=NTAGS),
                    )
                # emit
                nc.vector.scalar_tensor_tensor(
                    out=junks[x % 8][:], in0=iota[:], scalar=tagt[:, x:x + 1],
                    in1=emch[:, sub * NTAGS:(sub + 1) * NTAGS], op0=AL.is_equal, op1=AL.mult,
                    accum_out=eacc[:, x:x + 1],
                )
                # trans chain
                tb = ps.tile([P, P], F32, name=f"tb{x % 3}", tag=f"tb{x % 3}")
                nc.tensor.transpose(out=tb[:],
                                    in_=tagt[:, x:x + 1].to_broadcast([P, P]),
                                    identity=ident[:])
                oht = ohts[x % 4]
                nc.vector.tensor_scalar(out=oht[:], in0=tb[:], scalar1=iop[:],
                                        scalar2=None, op0=AL.is_equal)
                fp = ps.tile([P, P], F32, name=f"fp{x % 3}", tag=f"fp{x % 3}")
                nc.tensor.matmul(out=fp[:], lhsT=oht[:], rhs=tmat[:],
                                 start=True, stop=True)
                nc.vector.scalar_tensor_tensor(
                    out=junks[(x + 4) % 8][:], in0=iota[:], scalar=tagnt[:, x:x + 1],
                    in1=fp[:], op0=AL.is_equal, op1=AL.mult,
                    accum_out=tacc[:, x:x + 1],
                )

            # ---- epilogue: contrib = eacc*memt + tacc*mtrt, reduce over x//4
            c1 = sb.tile([P, 256], F32, name="c1")
            nc.vector.tensor_tensor(out=c1[:], in0=eacc[:], in1=memt[:], op=AL.mult)
            c2 = sb.tile([P, 256], F32, name="c2")
            nc.vector.tensor_tensor(out=c2[:], in0=tacc[:], in1=mtrt[:], op=AL.mult)
            cs = sb.tile([P, 256], F32, name="cs")
            nc.vector.tensor_tensor(out=cs[:], in0=c1[:], in1=c2[:], op=AL.add)
            part = sb.tile([P, 4], F32, name="part")
            nc.vector.tensor_reduce(
                out=part[:],
                in_=cs[:].rearrange("p (u t) -> p t u", t=4),
                axis=mybir.AxisListType.X, op=AL.add,
            )

            # ---- start term: SV[q, j] = startv[tag[0, b]], b = 128j+q
            sidx = sb.tile([P, 4], I32, name="sidx")
            nc.vector.tensor_copy(out=sidx[:], in_=tagt[:, 0:4])
            sv = sb.tile([P, 4], F32, name="sv")
            for j in range(4):
                nc.gpsimd.indirect_dma_start(
                    out=sv[:, j:j + 1], out_offset=None, in_=startv[:],
                    in_offset=bass.IndirectOffsetOnAxis(ap=sidx[:, j:j + 1], axis=0),
                )

            # ---- end term (exact): mask col-sums -> last tag -> endv
            mf = sb.tile([P, 2048], I32, name="mf")
            nc.sync.dma_start(out=mf[:],
                              in_=maskf_i[:].rearrange("(p f) -> p f", p=P))
            mff = sb.tile([P, 2048], F32, name="mff")
            nc.vector.tensor_copy(out=mff[:], in_=mf[:])
            ms1 = sb.tile([P, 512], F32, name="ms1")
            nc.vector.tensor_reduce(
                out=ms1[:],
                in_=mff[:].rearrange("p (sl b) -> p b sl", b=512),
                axis=mybir.AxisListType.X, op=AL.add,
            )
            msq = sb.tile([P, 4], F32, name="msq")
            for j in range(4):
                mp = ps.tile([P, 1], F32, name=f"mp{j}", tag="tb0")
                nc.tensor.matmul(out=mp[:], lhsT=ms1[:, j * P:(j + 1) * P],
                                 rhs=ones[:, 0:1], start=True, stop=True)
                nc.vector.tensor_copy(out=msq[:, j:j + 1], in_=mp[:])
            # si = ((msq - 1) * 512 + b) * 2,  b = 128j + q
            iop2_i = sb.tile([P, 1], I32, name="iop2_i")
            nc.gpsimd.iota(iop2_i[:], pattern=[[0, 1]], base=0, channel_multiplier=2)
            iop2 = sb.tile([P, 1], F32, name="iop2")
            nc.vector.tensor_copy(out=iop2[:], in_=iop2_i[:])
            j256_i = sb.tile([P, 4], I32, name="j256_i")
            nc.gpsimd.iota(j256_i[:], pattern=[[256, 4]], base=0, channel_multiplier=0)
            j256 = sb.tile([P, 4], F32, name="j256")
            nc.vector.tensor_copy(out=j256[:], in_=j256_i[:])
            si_f = sb.tile([P, 4], F32, name="si_f")
            nc.vector.tensor_scalar(out=si_f[:], in0=msq[:], scalar1=1024.0,
                                    scalar2=-1024.0, op0=AL.mult, op1=AL.add)
            nc.vector.tensor_scalar(out=si_f[:], in0=si_f[:], scalar1=iop2[:],
                                    scalar2=None, op0=AL.add)
            nc.vector.tensor_tensor(out=si_f[:], in0=si_f[:], in1=j256[:], op=AL.add)
            si4 = sb.tile([P, 4], I32, name="si4")
            nc.vector.tensor_copy(out=si4[:], in_=si_f[:])
            lt = sb.tile([P, 4], I32, name="lt")
            for j in range(4):
                nc.gpsimd.indirect_dma_start(
                    out=lt[:, j:j + 1], out_offset=None, in_=tagf_i[:],
                    in_offset=bass.IndirectOffsetOnAxis(ap=si4[:, j:j + 1], axis=0),
                )
            ev = sb.tile([P, 4], F32, name="ev")
            for j in range(4):
                nc.gpsimd.indirect_dma_start(
                    out=ev[:, j:j + 1], out_offset=None, in_=endv[:],
                    in_offset=bass.IndirectOffsetOnAxis(ap=lt[:, j:j + 1], axis=0),
                )

            # ---- total
            score = sb.tile([P, 4], F32, name="score")
            nc.vector.tensor_tensor(out=score[:], in0=part[:], in1=sv[:], op=AL.add)
            nc.vector.tensor_tensor(out=score[:], in0=score[:], in1=ev[:], op=AL.add)
            nc.sync.dma_start(out=out[:], in_=score[:])

    return nc


# ---------------------------------------------------------------------------
def _make_runner(nc, n_cores=8):
    import jax
    from jax.sharding import Mesh, PartitionSpec
    from jax.experimental.shard_map import shard_map
    import concourse.mybir as mybir
    from concourse import bass2jax

    bass2jax.install_neuronx_cc_hook()
    partition_name = nc.partition_id_tensor.name if nc.partition_id_tensor else None
    in_names, out_names, out_avals, zero_outs = [], [], [], []
    for alloc in nc.m.functions[0].allocations:
        if not isinstance(alloc, mybir.MemoryLocationSet):
            continue
        name = alloc.memorylocations[0].name
        if alloc.kind == "ExternalInput":
            if name != partition_name:
                in_names.append(name)
        elif alloc.kind == "ExternalOutput":
            shape = tuple(alloc.tensor_shape)
            dtype = mybir.dt.np(alloc.dtype)
            out_names.append(name)
            out_avals.append(jax.core.ShapedArray(shape, dtype))
            zero_outs.append(np.zeros(shape, dtype))
    n_params = len(in_names)
    all_in_names = list(in_names) + list(out_names)
    if partition_name is not None:
        all_in_names.append(partition_name)

    def _body(*args):
        operands = list(args)
        if partition_name is not None:
            operands.append(bass2jax.partition_id_tensor())
        outs = bass2jax._bass_exec_p.bind(
            *operands, out_avals=tuple(out_avals), in_names=tuple(all_in_names),
            out_names=tuple(out_names), lowering_input_output_aliases=(),
            sim_require_finite=True, sim_require_nnan=True, nc=nc,
        )
        return tuple(outs)

    devices = jax.devices()[:n_cores]
    mesh = Mesh(np.asarray(devices), ("core",))
    n_outs = len(out_names)
    jitted = jax.jit(
        shard_map(_body, mesh=mesh,
                  in_specs=(PartitionSpec("core"),) * (n_params + n_outs),
                  out_specs=(PartitionSpec("core"),) * n_outs, check_rep=False),
        keep_unused=True,
    )

    def run(in_maps):
        per_core = [[np.asarray(m[nm]) for nm in in_names] for m in in_maps]
        concat_in = [np.concatenate([per_core[c][i] for c in range(n_cores)], axis=0)
                     for i in range(n_params)]
        concat_zero = [np.concatenate([z] * n_cores, axis=0) for z in zero_outs]
        outs = [np.asarray(o) for o in jitted(*concat_in, *concat_zero)]
        results = []
        for c in range(n_cores):
            d = {}
            for i, nm in enumerate(out_names):
                per = outs[i].shape[0] // n_cores
                d[nm] = outs[i][c * per:(c + 1) * per]
            results.append(d)
        return results

    return run


def _get_runner():
    global _RUNNER
    if _RUNNER is None:
        _install_tile_patch()
        _RUNNER = _make_runner(_build_nc(), NCORES)
    return _RUNNER


# ---------------------------------------------------------------------------
def make_in_maps(emissions, tags, mask, start_transitions, end_transitions,
                 transitions):
    import ml_dtypes

    emissions = np.ascontiguousarray(emissions, dtype=np.float32)
    tags = np.ascontiguousarray(tags, dtype=np.int64)
    mask = np.ascontiguousarray(mask, dtype=np.int32)
    tmat_bf = np.ascontiguousarray(
        transitions.astype(ml_dtypes.bfloat16))
    startv = np.ascontiguousarray(start_transitions, np.float32).reshape(NTAGS, 1)
    endv = np.ascontiguousarray(end_transitions, np.float32).reshape(NTAGS, 1)
    maskf_i = np.ascontiguousarray(mask, np.int32).reshape(-1)
    tagf_i = tags.view(np.int32).reshape(-1, 1).copy()

    in_maps = []
    for k in range(NCORES):
        s0 = k * SLICE
        em_k = emissions[s0:s0 + SLICE].reshape(-1)
        tag_k = np.ascontiguousarray(tags[s0:s0 + SLICE]).view(np.int32).reshape(-1)
        if k < NCORES - 1:
            tagn_k = np.ascontiguousarray(tags[s0 + 1:s0 + SLICE + 1]).view(np.int32).reshape(-1)
            masktr_k = np.ascontiguousarray(mask[s0 + 1:s0 + SLICE + 1]).reshape(-1)
        else:
            tagn_k = np.ascontiguousarray(
                np.concatenate([tags[s0 + 1:], tags[-1:]])).view(np.int32).reshape(-1)
            masktr_k = np.concatenate(
                [mask[s0 + 1:], np.zeros((1, BATCH), np.int32)]).reshape(-1)
        maskem_k = mask[s0:s0 + SLICE].copy()
        if k == 0:
            maskem_k[0, :] = 1
        zero128 = np.zeros((NTAGS, 1), np.float32)
        in_maps.append({
            "em": em_k,
            "tagx_i": tag_k,
            "tagnx_i": tagn_k,
            "maskem_i": maskem_k.reshape(-1),
            "masktr_i": np.ascontiguousarray(masktr_k, np.int32),
            "tmat_bf": tmat_bf,
            "startv": startv if k == 0 else zero128,
            "endv": endv if k == NCORES - 1 else zero128,
            "maskf_i": maskf_i,
            "tagf_i": tagf_i,
        })
    return in_maps


def kernel(emissions, tags, mask, start_transitions, end_transitions,
           transitions):
    run = _get_runner()
    in_maps = make_in_maps(emissions, tags, mask, start_transitions,
                           end_transitions, transitions)
    results = run(in_maps)
    total = np.zeros((P, 4), np.float64)
    for r in results:
        total += r["out"].astype(np.float64)
    score = total.T.reshape(BATCH).astype(np.float32)
    return score


# revision 8
# speedup vs baseline: 12793.8703x; 12793.8703x over previous
"""CRF sequence-score kernel for Trainium2 (8 NeuronCores, SPMD).

Strategy (S-shard: core k owns s in [64k, 64k+64), all 512 batches):
  rows r = s_local*512 + b, laid out as [q = r%128 partitions, x = r//128].
  - emit[r] = emissions[r, tags[r]] via one fused DVE scalar_tensor_tensor
    per 128-row block: accum_out = sum_t (iota_t == tag) * em[r, t].
  - trans[r] = T[tag_r, tagnext_r] via PE chain per block: broadcast-matmul
    of the block's tag row -> transposed one-hot (tensor_scalar vs partition
    iota) -> matmul fetches T rows into PSUM -> same stt selects tagnext.
  - masks folded in a small epilogue; reduction over s via AP-strided
    tensor_reduce; start/end terms via 1-idx-per-partition indirect DMA
    (end term computed exactly: mask column-sum -> last tag gather -> end
    table gather).
Host sums the 8 per-core [128, 4] partials; score[b] = total[b%128, b//128].
"""
import numpy as np

SEQ, BATCH, NTAGS = 512, 512, 128
NCORES = 8
SLICE = SEQ // NCORES            # 64 s-rows per core
NROWS = SLICE * BATCH            # 32768 rows per core
NBLK = NROWS // 128              # 256 blocks of 128 rows
P = 128

_RUNNER = None


# ---------------------------------------------------------------------------
# walrus workaround: this build allows only ONE sync-wait per instruction.
def _install_tile_patch():
    import bass_rust
    import concourse.mybir as mybir
    import concourse.tile as tile
    from concourse.vector_clock import ScopedClock

    if getattr(tile.TileContext, "_crf_patched", False):
        return

    def _drain_and_barrier(self, tick_clock, wait_clock):
        nc = self.nc
        drain_inst = nc.sync.drain()
        wait_clock.add_sem_waits(
            drain_inst.ins, ScopedClock({None: tick_clock.global_clock})
        )
        si = drain_inst.ins.sync_info
        waits = list(si.on_wait) if si is not None and si.on_wait else []
        if len(waits) > 1:
            si.on_wait = waits[:1]
            for w in waits[1:]:
                extra = nc.sync.drain()
                if extra.ins.sync_info is None:
                    extra.ins.sync_info = bass_rust.SyncInfo(on_wait=[], on_update=[])
                extra.ins.sync_info.on_wait = [w]
        nc.all_engine_barrier()
        assert self.sems is not None
        popped = nc._tile_sem_poison_stack.pop()
        assert popped is self._sem_poison
        nc.clear_and_free_semaphores(list(self.sems.allocated().values()))
        nc.all_engine_barrier()

    orig_commit = tile.TileContext._commit_instruction

    def _commit(self, inst, lazy_reg_writes=True):
        si = getattr(inst, "sync_info", None)
        if (
            si is not None
            and si.on_wait
            and len(si.on_wait) > 1
            and inst.engine != mybir.EngineType.Unassigned
        ):
            waits = list(si.on_wait)
            si.on_wait = waits[:1]
            for w in waits[1:]:
                nop = mybir.InstNoOp(name=f"I-{self.nc.next_id()}", ins=[], outs=[])
                nop.engine = inst.engine
                nop.sync_info = bass_rust.SyncInfo(on_wait=[w], on_update=[])
                self._add_instruction(nop)
        return orig_commit(self, inst, lazy_reg_writes)

    tile.TileContext._drain_and_barrier = _drain_and_barrier
    tile.TileContext._commit_instruction = _commit
    tile.TileContext._crf_patched = True


# ---------------------------------------------------------------------------
def _build_nc(skip_main=False, no_trans=False):
    import concourse.bass as bass
    import concourse.mybir as mybir
    import concourse.tile as tile
    from concourse.masks import make_identity

    F32, I32, BF16, I16 = (mybir.dt.float32, mybir.dt.int32,
                           mybir.dt.bfloat16, mybir.dt.int16)
    AL = mybir.AluOpType

    nc = bass.Bass()
    em = nc.declare_dram_parameter("em", [NROWS * NTAGS], F32, isOutput=False)
    tagx_i = nc.declare_dram_parameter("tagx_i", [NROWS * 2], I32, isOutput=False)
    tagnx_i = nc.declare_dram_parameter("tagnx_i", [NROWS * 2], I32, isOutput=False)
    maskem_i = nc.declare_dram_parameter("maskem_i", [NROWS], I32, isOutput=False)
    masktr_i = nc.declare_dram_parameter("masktr_i", [NROWS], I32, isOutput=False)
    tmat_bf = nc.declare_dram_parameter("tmat_bf", [P, NTAGS], BF16, isOutput=False)
    startv = nc.declare_dram_parameter("startv", [NTAGS, 1], F32, isOutput=False)
    endv = nc.declare_dram_parameter("endv", [NTAGS, 1], F32, isOutput=False)
    maskf_i = nc.declare_dram_parameter("maskf_i", [SEQ * BATCH], I32, isOutput=False)
    tagf_i = nc.declare_dram_parameter("tagf_i", [SEQ * BATCH * 2, 1], I32, isOutput=False)
    out = nc.declare_dram_parameter("out", [P, 4], F32, isOutput=True)

    with tile.TileContext(nc) as tc:
        with tc.tile_pool(name="sbuf", bufs=1) as sb, \
             tc.tile_pool(name="psum", bufs=1, space="PSUM") as ps, \
             tc.tile_pool(name="emp", bufs=3) as emp:
            # ---- constants
            iota_i = sb.tile([P, NTAGS], I32, name="iota_i")
            nc.gpsimd.iota(iota_i[:], pattern=[[1, NTAGS]], base=0, channel_multiplier=0)
            iota = sb.tile([P, NTAGS], F32, name="iota")
            nc.vector.tensor_copy(out=iota[:], in_=iota_i[:])
            iop_i = sb.tile([P, 1], I32, name="iop_i")
            nc.gpsimd.iota(iop_i[:], pattern=[[0, 1]], base=0, channel_multiplier=1)
            iop = sb.tile([P, 1], F32, name="iop")
            nc.vector.tensor_copy(out=iop[:], in_=iop_i[:])
            niop = sb.tile([P, 1], F32, name="niop")
            nc.vector.tensor_scalar(out=niop[:], in0=iop[:], scalar1=-1.0,
                                    scalar2=None, op0=AL.mult)
            ones = sb.tile([P, P], F32, name="ones")
            nc.vector.memset(ones[:], 1.0)
            ident = sb.tile([P, P], F32, name="ident")
            make_identity(nc, ident[:])

            # ---- T matrix (bf16) stationary
            tmat = sb.tile([P, NTAGS], BF16, name="tmat")
            nc.sync.dma_start(out=tmat[:], in_=tmat_bf[:])

            # ---- tag/mask staging: x-major loads -> f32 -> PE transpose
            # TAGX[p, sub*128+m] = tag[128*(sub*128+p) + m]
            def stage_tags(name, dram):
                raw = sb.tile([P, 512], I32, name=f"{name}_raw")
                nc.sync.dma_start(
                    out=raw[:].rearrange("p (s i) -> p s i", s=2),
                    in_=dram[:].rearrange("(s p i) -> p s i", s=2, p=P, i=256),
                )
                f = sb.tile([P, 256], F32, name=f"{name}_f")
                nc.vector.tensor_copy(
                    out=f[:].rearrange("p (s m) -> p s m", s=2),
                    in_=raw[:].rearrange("p (s m two) -> p s m two", s=2, two=2)[:, :, :, 0:1],
                )
                return f

            def stage_mask(name, dram):
                raw = sb.tile([P, 256], I32, name=f"{name}_raw")
                nc.sync.dma_start(
                    out=raw[:].rearrange("p (s i) -> p s i", s=2),
                    in_=dram[:].rearrange("(s p i) -> p s i", s=2, p=P, i=P),
                )
                f = sb.tile([P, 256], F32, name=f"{name}_f")
                nc.vector.tensor_copy(out=f[:], in_=raw[:])
                return f

            tagx = stage_tags("tagx", tagx_i)     # [128, 2, 128] f32
            tagnx = stage_tags("tagnx", tagnx_i)
            mex = stage_mask("mex", maskem_i)
            mtx = stage_mask("mtx", masktr_i)

            # transpose halves -> [q, x] layout [128, 256]
            def transpose_qx(name, src):
                dst = sb.tile([P, 256], F32, name=f"{name}_t")
                for h in range(2):
                    tp = ps.tile([P, P], F32, name=f"{name}_tp{h}", tag=f"tb{h}")
                    nc.tensor.transpose(out=tp[:], in_=src[:, h * P:(h + 1) * P],
                                        identity=ident[:])
                    nc.scalar.copy(out=dst[:, h * P:(h + 1) * P], in_=tp[:])
                return dst

            tagt = transpose_qx("tagt", tagx)     # tag in [q, x]
            tagnt = transpose_qx("tagnt", tagnx)  # tagnext in [q, x]
            memt = transpose_qx("memt", mex)      # maskEM in [q, x]
            mtrt = transpose_qx("mtrt", mtx)      # maskTR in [q, x]

            # ---- main loop: emit-stt + trans chain per block x
            eacc = sb.tile([P, 256], F32, name="eacc")
            tacc = sb.tile([P, 256], F32, name="tacc")
            if skip_main:
                nc.vector.memset(eacc[:], 0.0)
            if skip_main or no_trans:
                nc.vector.memset(tacc[:], 0.0)
            junks = [sb.tile([P, NTAGS], F32, name=f"junk{i}", tag=f"jk{i}")
                     for i in range(16)]
            ohts = [sb.tile([P, P], BF16, name=f"oht{i}", tag=f"oh{i}")
                    for i in range(8)]
            d2s = [sb.tile([P, P], F32, name=f"d2_{i}", tag=f"d2{i}")
                   for i in range(8)]
            emch = None
            for x in range(0 if skip_main else NBLK):
                d, sub = x // 16, x % 16
                if sub == 0:
                    emch = emp.tile([P, 16 * NTAGS], F32, name=f"emch{d}", tag="emch")
                    nc.sync.dma_start(
                        out=emch[:].rearrange("p (s t) -> p s t", s=16),
                        in_=em[d * 16 * 16384:(d + 1) * 16 * 16384].rearrange(
                            "(s p t) -> p s t", s=16, p=P, t=NTAGS),
                    )
                # emit
                nc.vector.scalar_tensor_tensor(
                    out=junks[x % 16][:], in0=iota[:], scalar=tagt[:, x:x + 1],
                    in1=emch[:, sub * NTAGS:(sub + 1) * NTAGS], op0=AL.is_equal, op1=AL.mult,
                    accum_out=eacc[:, x:x + 1],
                )
                # trans chain
                if no_trans:
                    continue
                tb = ps.tile([P, P], F32, name=f"tb{x % 4}", tag=f"tb{x % 4}")
                nc.tensor.transpose(out=tb[:],
                                    in_=tagt[:, x:x + 1].to_broadcast([P, P]),
                                    identity=ident[:])
                oht = ohts[x % 8]
                d2 = d2s[x % 8]
                nc.scalar.activation(out=d2[:], in_=tb[:],
                                     func=mybir.ActivationFunctionType.Square,
                                     bias=niop[:], scale=1.0)
                nc.scalar.activation(out=oht[:], in_=d2[:],
                                     func=mybir.ActivationFunctionType.Relu,
                                     bias=1.0, scale=-1.0)
                fp = ps.tile([P, P], F32, name=f"fp{x % 4}", tag=f"fp{x % 4}")
                nc.tensor.matmul(out=fp[:], lhsT=oht[:], rhs=tmat[:],
                                 start=True, stop=True)
                nc.vector.scalar_tensor_tensor(
                    out=junks[(x + 8) % 16][:], in0=iota[:], scalar=tagnt[:, x:x + 1],
                    in1=fp[:], op0=AL.is_equal, op1=AL.mult,
                    accum_out=tacc[:, x:x + 1],
                )

            # ---- epilogue: contrib = eacc*memt + tacc*mtrt, reduce over x//4
            c1 = sb.tile([P, 256], F32, name="c1")
            nc.vector.tensor_tensor(out=c1[:], in0=eacc[:], in1=memt[:], op=AL.mult)
            c2 = sb.tile([P, 256], F32, name="c2")
            nc.vector.tensor_tensor(out=c2[:], in0=tacc[:], in1=mtrt[:], op=AL.mult)
            cs = sb.tile([P, 256], F32, name="cs")
            nc.vector.tensor_tensor(out=cs[:], in0=c1[:], in1=c2[:], op=AL.add)
            part = sb.tile([P, 4], F32, name="part")
            nc.vector.tensor_reduce(
                out=part[:],
                in_=cs[:].rearrange("p (u t) -> p t u", t=4),
                axis=mybir.AxisListType.X, op=AL.add,
            )

            # ---- start term: SV[q, j] = startv[tag[0, b]], b = 128j+q
            sidx = sb.tile([P, 4], I32, name="sidx")
            nc.vector.tensor_copy(out=sidx[:], in_=tagt[:, 0:4])
            sv = sb.tile([P, 4], F32, name="sv")
            for j in range(4):
                nc.gpsimd.indirect_dma_start(
                    out=sv[:, j:j + 1], out_offset=None, in_=startv[:],
                    in_offset=bass.IndirectOffsetOnAxis(ap=sidx[:, j:j + 1], axis=0),
                )

            # ---- end term (exact): mask col-sums -> last tag -> endv
            mf = sb.tile([P, 2048], I32, name="mf")
            nc.sync.dma_start(out=mf[:],
                              in_=maskf_i[:].rearrange("(p f) -> p f", p=P))
            mff = sb.tile([P, 2048], F32, name="mff")
            nc.vector.tensor_copy(out=mff[:], in_=mf[:])
            ms1 = sb.tile([P, 512], F32, name="ms1")
            nc.vector.tensor_reduce(
                out=ms1[:],
                in_=mff[:].rearrange("p (sl b) -> p b sl", b=512),
                axis=mybir.AxisListType.X, op=AL.add,
            )
            msq = sb.tile([P, 4], F32, name="msq")
            for j in range(4):
                mp = ps.tile([P, 1], F32, name=f"mp{j}", tag="tb0")
                nc.tensor.matmul(out=mp[:], lhsT=ms1[:, j * P:(j + 1) * P],
                                 rhs=ones[:, 0:1], start=True, stop=True)
                nc.vector.tensor_copy(out=msq[:, j:j + 1], in_=mp[:])
            # si = ((msq - 1) * 512 + b) * 2,  b = 128j + q
            iop2_i = sb.tile([P, 1], I32, name="iop2_i")
            nc.gpsimd.iota(iop2_i[:], pattern=[[0, 1]], base=0, channel_multiplier=2)
            iop2 = sb.tile([P, 1], F32, name="iop2")
            nc.vector.tensor_copy(out=iop2[:], in_=iop2_i[:])
            j256_i = sb.tile([P, 4], I32, name="j256_i")
            nc.gpsimd.iota(j256_i[:], pattern=[[256, 4]], base=0, channel_multiplier=0)
            j256 = sb.tile([P, 4], F32, name="j256")
            nc.vector.tensor_copy(out=j256[:], in_=j256_i[:])
            si_f = sb.tile([P, 4], F32, name="si_f")
            nc.vector.tensor_scalar(out=si_f[:], in0=msq[:], scalar1=1024.0,
                                    scalar2=-1024.0, op0=AL.mult, op1=AL.add)
            nc.vector.tensor_scalar(out=si_f[:], in0=si_f[:], scalar1=iop2[:],
                                    scalar2=None, op0=AL.add)
            nc.vector.tensor_tensor(out=si_f[:], in0=si_f[:], in1=j256[:], op=AL.add)
            si4 = sb.tile([P, 4], I32, name="si4")
            nc.vector.tensor_copy(out=si4[:], in_=si_f[:])
            lt = sb.tile([P, 4], I32, name="lt")
            for j in range(4):
                nc.gpsimd.indirect_dma_start(
                    out=lt[:, j:j + 1], out_offset=None, in_=tagf_i[:],
                    in_offset=bass.IndirectOffsetOnAxis(ap=si4[:, j:j + 1], axis=0),
                )
            ev = sb.tile([P, 4], F32, name="ev")
            for j in range(4):
                nc.gpsimd.indirect_dma_start(
                    out=ev[:, j:j + 1], out_offset=None, in_=endv[:],
                    in_offset=bass.IndirectOffsetOnAxis(ap=lt[:, j:j + 1], axis=0),
                )

            # ---- total
            score = sb.tile([P, 4], F32, name="score")
            nc.vector.tensor_tensor(out=score[:], in0=part[:], in1=sv[:], op=AL.add)
            nc.vector.tensor_tensor(out=score[:], in0=score[:], in1=ev[:], op=AL.add)
            nc.sync.dma_start(out=out[:], in_=score[:])

    return nc


# ---------------------------------------------------------------------------
def _make_runner(nc, n_cores=8):
    import jax
    from jax.sharding import Mesh, PartitionSpec
    from jax.experimental.shard_map import shard_map
    import concourse.mybir as mybir
    from concourse import bass2jax

    bass2jax.install_neuronx_cc_hook()
    partition_name = nc.partition_id_tensor.name if nc.partition_id_tensor else None
    in_names, out_names, out_avals, zero_outs = [], [], [], []
    for alloc in nc.m.functions[0].allocations:
        if not isinstance(alloc, mybir.MemoryLocationSet):
            continue
        name = alloc.memorylocations[0].name
        if alloc.kind == "ExternalInput":
            if name != partition_name:
                in_names.append(name)
        elif alloc.kind == "ExternalOutput":
            shape = tuple(alloc.tensor_shape)
            dtype = mybir.dt.np(alloc.dtype)
            out_names.append(name)
            out_avals.append(jax.core.ShapedArray(shape, dtype))
            zero_outs.append(np.zeros(shape, dtype))
    n_params = len(in_names)
    all_in_names = list(in_names) + list(out_names)
    if partition_name is not None:
        all_in_names.append(partition_name)

    def _body(*args):
        operands = list(args)
        if partition_name is not None:
            operands.append(bass2jax.partition_id_tensor())
        outs = bass2jax._bass_exec_p.bind(
            *operands, out_avals=tuple(out_avals), in_names=tuple(all_in_names),
            out_names=tuple(out_names), lowering_input_output_aliases=(),
            sim_require_finite=True, sim_require_nnan=True, nc=nc,
        )
        return tuple(outs)

    devices = jax.devices()[:n_cores]
    mesh = Mesh(np.asarray(devices), ("core",))
    n_outs = len(out_names)
    jitted = jax.jit(
        shard_map(_body, mesh=mesh,
                  in_specs=(PartitionSpec("core"),) * (n_params + n_outs),
                  out_specs=(PartitionSpec("core"),) * n_outs, check_rep=False),
        keep_unused=True,
    )

    def run(in_maps):
        per_core = [[np.asarray(m[nm]) for nm in in_names] for m in in_maps]
        concat_in = [np.concatenate([per_core[c][i] for c in range(n_cores)], axis=0)
                     for i in range(n_params)]
        concat_zero = [np.concatenate([z] * n_cores, axis=0) for z in zero_outs]
        outs = [np.asarray(o) for o in jitted(*concat_in, *concat_zero)]
        results = []
        for c in range(n_cores):
            d = {}
            for i, nm in enumerate(out_names):
                per = outs[i].shape[0] // n_cores
                d[nm] = outs[i][c * per:(c + 1) * per]
            results.append(d)
        return results

    return run


def _get_runner():
    global _RUNNER
    if _RUNNER is None:
        _install_tile_patch()
        _RUNNER = _make_runner(_build_nc(), NCORES)
    return _RUNNER


# ---------------------------------------------------------------------------
def make_in_maps(emissions, tags, mask, start_transitions, end_transitions,
                 transitions):
    import ml_dtypes

    emissions = np.ascontiguousarray(emissions, dtype=np.float32)
    tags = np.ascontiguousarray(tags, dtype=np.int64)
    mask = np.ascontiguousarray(mask, dtype=np.int32)
    tmat_bf = np.ascontiguousarray(
        transitions.astype(ml_dtypes.bfloat16))
    startv = np.ascontiguousarray(start_transitions, np.float32).reshape(NTAGS, 1)
    endv = np.ascontiguousarray(end_transitions, np.float32).reshape(NTAGS, 1)
    maskf_i = np.ascontiguousarray(mask, np.int32).reshape(-1)
    tagf_i = tags.view(np.int32).reshape(-1, 1).copy()

    in_maps = []
    for k in range(NCORES):
        s0 = k * SLICE
        em_k = emissions[s0:s0 + SLICE].reshape(-1)
        tag_k = np.ascontiguousarray(tags[s0:s0 + SLICE]).view(np.int32).reshape(-1)
        if k < NCORES - 1:
            tagn_k = np.ascontiguousarray(tags[s0 + 1:s0 + SLICE + 1]).view(np.int32).reshape(-1)
            masktr_k = np.ascontiguousarray(mask[s0 + 1:s0 + SLICE + 1]).reshape(-1)
        else:
            tagn_k = np.ascontiguousarray(
                np.concatenate([tags[s0 + 1:], tags[-1:]])).view(np.int32).reshape(-1)
            masktr_k = np.concatenate(
                [mask[s0 + 1:], np.zeros((1, BATCH), np.int32)]).reshape(-1)
        maskem_k = mask[s0:s0 + SLICE].copy()
        if k == 0:
            maskem_k[0, :] = 1
        zero128 = np.zeros((NTAGS, 1), np.float32)
        in_maps.append({
            "em": em_k,
            "tagx_i": tag_k,
            "tagnx_i": tagn_k,
            "maskem_i": maskem_k.reshape(-1),
            "masktr_i": np.ascontiguousarray(masktr_k, np.int32),
            "tmat_bf": tmat_bf,
            "startv": startv if k == 0 else zero128,
            "endv": endv if k == NCORES - 1 else zero128,
            "maskf_i": maskf_i,
            "tagf_i": tagf_i,
        })
    return in_maps


def kernel(emissions, tags, mask, start_transitions, end_transitions,
           transitions):
    run = _get_runner()
    in_maps = make_in_maps(emissions, tags, mask, start_transitions,
                           end_transitions, transitions)
    results = run(in_maps)
    total = np.zeros((P, 4), np.float64)
    for r in results:
        total += r["out"].astype(np.float64)
    score = total.T.reshape(BATCH).astype(np.float32)
    return score
